# revision 1
# baseline (speedup 1.0000x reference)
"""Trainium2 Bass kernel for nn_CombinedLoss (Poisson + 3-way pairwise CLIP loss).

Strategy (8 NeuronCores, SPMD, no collectives):
  - Row-shard the batch: core c owns rows [c*512, (c+1)*512) of every tensor.
  - For each feature pair (a,b) in {(1,2),(1,3),(2,3)} each core computes its
    512x4096 block of S_ab = Za @ Zb^T with bf16 matmuls (fp32 PSUM accum):
      lhsT = raw-cast bf16 own-slice of a, transposed on-chip (PE transpose);
      rhs  = normalized bf16 full feature b, transposed via DMA xbar
             (bf16 roundtrip through a DRAM scratch buffer).
    The 1/||a|| normalization of the lhsT side is folded into the exp's
    per-partition scale on the Scalar engine: exp(S_raw * (2/||a_m||)).
  - Row-wise sum(exp) comes free via the activation's accum_out.
  - Column-wise sum(exp) via ones-vector matmuls (contraction over partitions),
    accumulated in PSUM across the 4 M-tiles; partial per-core, host combines.
  - 1/sqrt on device via bit-trick + 2 Newton steps on the Vector engine
    (avoids ACT Ln/Exp table thrashing; ACT does only Exp + poisson Ln).
  - Diagonal similarities via fused multiply+reduce on own slices (raw dots,
    normalized on host with the device-computed squared norms).
  - Host does only the O(B) final combine: log of 4096-length sums, means.
"""

import os
import sys

import numpy as np

sys.path.insert(0, "/opt/trn_rl_repo")

P = 128
TEMPERATURE = 0.5
EPS_POISSON = 1e-8
RSQRT_MAGIC = 0x5F3759DF


class Cfg:
    def __init__(self, B=4096, D=1024, n_cores=8, ntc=512):
        self.B = B          # batch
        self.D = D          # feature dim
        self.n_cores = n_cores
        self.S = B // n_cores      # own rows per core
        self.MT = self.S // P      # M tiles (own rows / 128)
        self.K = D // P            # contraction tiles
        self.NTC = ntc             # columns per rhs tile
        self.NT = B // ntc         # number of rhs tiles
        self.ST = ntc // P         # row-subtiles per rhs tile
        assert B % n_cores == 0 and self.S % P == 0 and D % P == 0 and B % ntc == 0


def _patch_act_tables():
    """Make Bacc's act-table pass pick `natural_log_exp_and_others` for both
    Exp and Ln (they otherwise land in two different sets, and alternating
    Ln/Exp calls reload the 2.7us activation tables every tile).

    Keeps list order (index == act_func_set_id) but empties the earlier
    exp-only / ln-only sets so the first set containing Exp or Ln is the
    combined one."""
    import functools

    import concourse.hw_specs as hw_specs

    if getattr(hw_specs, "_act_tables_patched", False):
        return
    orig = hw_specs.get_activation_tables

    @functools.cache
    def patched(module_arch):
        tabs = dict(orig(module_arch))
        names = list(tabs.keys())
        if "natural_log_exp_and_others" in tabs:
            combined = tabs["natural_log_exp_and_others"]
            for name in names:
                if name == "natural_log_exp_and_others":
                    break
                if tabs[name] & combined:
                    tabs[name] = tabs[name] - combined
        return tabs

    hw_specs.get_activation_tables = patched
    # bacc imports the symbol lazily via module attr? patch its ref if bound
    import concourse.bacc as bacc_mod

    if hasattr(bacc_mod, "get_activation_tables"):
        bacc_mod.get_activation_tables = patched
    hw_specs._act_tables_patched = True


def build_bass(cfg: Cfg):
    """Build the single-core Bass program (same program for all SPMD cores)."""
    import concourse.bacc as bacc
    import concourse.bass as bass
    import concourse.mybir as mybir
    import concourse.tile as tile
    from concourse.masks import make_identity

    _patch_act_tables()

    f32 = mybir.dt.float32
    bf16 = mybir.dt.bfloat16
    i32 = mybir.dt.int32
    AF = mybir.ActivationFunctionType
    ALU = mybir.AluOpType
    ts = bass.ts

    B, D, K, MT, NT, NTC, ST = cfg.B, cfg.D, cfg.K, cfg.MT, cfg.NT, cfg.NTC, cfg.ST

    nc = bacc.Bacc(
        "TRN2",
        target_bir_lowering=False,
        debug=False,
        enable_asserts=False,
        num_devices=cfg.n_cores,
    )

    # ---- IO ----
    f1o = nc.dram_tensor("f1_own", [cfg.S, D], f32, kind="ExternalInput").ap()
    f2o = nc.dram_tensor("f2_own", [cfg.S, D], f32, kind="ExternalInput").ap()
    f3o = nc.dram_tensor("f3_own", [cfg.S, D], f32, kind="ExternalInput").ap()
    f2f = nc.dram_tensor("f2_full", [B, D], f32, kind="ExternalInput").ap()
    f3f = nc.dram_tensor("f3_full", [B, D], f32, kind="ExternalInput").ap()
    inp = nc.dram_tensor("inp_own", [cfg.S, D], f32, kind="ExternalInput").ap()
    tgt = nc.dram_tensor("tgt_own", [cfg.S, D], f32, kind="ExternalInput").ap()

    rowparts_d = nc.dram_tensor("rowparts", [P, 3 * MT * NT], f32, kind="ExternalOutput").ap()
    colparts_d = nc.dram_tensor("colparts", [1, 3 * B], f32, kind="ExternalOutput").ap()
    nsq_d = nc.dram_tensor("nsq_own", [P, 3 * MT], f32, kind="ExternalOutput").ap()
    dots_d = nc.dram_tensor("dots_own", [P, 3 * MT], f32, kind="ExternalOutput").ap()
    poi_d = nc.dram_tensor("poi", [P, 2 * MT], f32, kind="ExternalOutput").ap()

    own_dram = [f1o, f2o, f3o]

    with tile.TileContext(nc) as tc:
        with (
            tc.tile_pool(name="const", bufs=1) as const_pool,
            tc.tile_pool(name="persist", bufs=1) as persist,
            tc.tile_pool(name="stage", bufs=6) as stage,
            tc.tile_pool(name="stage16", bufs=4) as stage16,
            tc.tile_pool(name="junk", bufs=2) as junkp,
            tc.tile_pool(name="rhs", bufs=3) as rhsp,
            tc.tile_pool(name="exps", bufs=5) as expp,
            tc.tile_pool(name="small", bufs=6) as smallp,
            tc.tile_pool(name="colpp", bufs=2) as colpp,
            tc.tile_pool(name="dscr", bufs=3, space="DRAM") as dramp,
            tc.tile_pool(name="ps_s", bufs=4, space="PSUM") as ps_s,
            tc.tile_pool(name="ps_t", bufs=2, space="PSUM") as ps_t,
            tc.tile_pool(name="ps_c", bufs=2, space="PSUM") as ps_c,
        ):
            identity = const_pool.tile([P, P], bf16)
            make_identity(nc, identity)
            ones = const_pool.tile([P, 1], bf16)
            nc.vector.memset(ones, 1.0)
            eps_bias = const_pool.tile([P, 1], f32)
            nc.vector.memset(eps_bias, EPS_POISSON)

            # persistent accumulators / stats
            zT1_own = persist.tile([P, K, cfg.S], bf16)
            zT2_own = persist.tile([P, K, cfg.S], bf16)
            rowparts = persist.tile([P, 3 * MT * NT], f32)
            nsq_own = persist.tile([P, 3 * MT], f32)
            dots_own = persist.tile([P, 3 * MT], f32)
            poi = persist.tile([P, 2 * MT], f32)
            scaleA = persist.tile([P, 2 * MT], f32)  # (1/T)/||a|| for f1, f2 own rows

            zT_own = [zT1_own, zT2_own]

            def rsqrt_act(dst, src, n, tag):
                # dst[:, :n] = 1/sqrt(src) = exp(-0.5*ln(src)); Ln and Exp share
                # one activation table set (patched below), so no table thrash.
                l = smallp.tile([P, n], f32, tag=tag)
                nc.scalar.activation(l, src, AF.Ln)
                nc.scalar.activation(dst, l, AF.Exp, scale=-0.5)

            def transpose_rowtile_pe(rb16, zT_dst, t):
                # rb16: [128 rows, D] bf16 row-major -> zT_dst[:, k, t*128:(t+1)*128]
                tps = ps_t.tile([P, K * P], bf16, tag="tps")
                for k in range(K):
                    nc.tensor.transpose(tps[:, ts(k, P)], rb16[:, ts(k, P)], identity)
                nc.any.tensor_copy(
                    out=zT_dst[:, :, ts(t, P)],
                    in_=tps[:].rearrange("p (k c) -> p k c", k=K),
                )

            # ---------------- Phase 0a: own f1/f2 (matmul-critical) ----------------
            own_rf = {}
            for t in range(MT):
                for fi in range(2):
                    rf = stage.tile([P, D], f32, tag="rowf32")
                    nc.sync.dma_start(rf, own_dram[fi][ts(t, P), :])
                    own_rf[(fi, t)] = rf
                    rb = stage16.tile([P, D], bf16, tag="rowbf16")
                    nc.vector.tensor_scalar_mul(rb, rf, 1.0)
                    jt = junkp.tile([P, D], bf16, tag="junk16")
                    nc.vector.scalar_tensor_tensor(
                        out=jt, in0=rb, scalar=1.0, in1=rb,
                        op0=ALU.mult, op1=ALU.mult,
                        accum_out=nsq_own[:, fi * MT + t : fi * MT + t + 1],
                    )
                    transpose_rowtile_pe(rb, zT_own[fi], t)

            # own-row exp scales: (1/T) * rsqrt(nsq) for f1, f2
            recip_own = smallp.tile([P, 2 * MT], f32, tag="recip_own")
            rsqrt_act(recip_own, nsq_own[:, : 2 * MT], 2 * MT, tag="ln_own")
            nc.vector.tensor_scalar_mul(scaleA, recip_own, 1.0 / TEMPERATURE)

            def phase0_tail():
                # f3 norms, raw diagonal dots, poisson partials (independent of
                # the matmul stream; emitted last to fill idle DVE/ACT time)
                for t in range(MT):
                    rfs = []
                    for fi in range(3):
                        rf = stage.tile([P, D], f32, tag="rowf32")
                        nc.sync.dma_start(rf, own_dram[fi][ts(t, P), :])
                        rfs.append(rf)
                    jt = junkp.tile([P, D], bf16, tag="junk16")
                    nc.vector.scalar_tensor_tensor(
                        out=jt, in0=rfs[2], scalar=1.0, in1=rfs[2],
                        op0=ALU.mult, op1=ALU.mult,
                        accum_out=nsq_own[:, 2 * MT + t : 2 * MT + t + 1],
                    )
                    for pi, (ia, ib) in enumerate(((0, 1), (0, 2), (1, 2))):
                        jt = junkp.tile([P, D], bf16, tag="junk16")
                        nc.vector.scalar_tensor_tensor(
                            out=jt, in0=rfs[ia], scalar=1.0, in1=rfs[ib],
                            op0=ALU.mult, op1=ALU.mult,
                            accum_out=dots_own[:, pi * MT + t : pi * MT + t + 1],
                        )
                    it = stage.tile([P, D], f32, tag="rowf32")
                    tt = stage.tile([P, D], f32, tag="rowf32")
                    nc.sync.dma_start(it, inp[ts(t, P), :])
                    nc.sync.dma_start(tt, tgt[ts(t, P), :])
                    lg = stage.tile([P, D], f32, tag="rowf32")
                    nc.scalar.activation(lg, it, AF.Ln, bias=eps_bias[:, :])
                    jt = junkp.tile([P, D], bf16, tag="junk16")
                    nc.vector.scalar_tensor_tensor(
                        out=jt, in0=tt, scalar=1.0, in1=lg,
                        op0=ALU.mult, op1=ALU.mult,
                        accum_out=poi[:, MT + t : MT + t + 1],
                    )
                    jt2 = junkp.tile([P, D], bf16, tag="junk16")
                    nc.vector.tensor_scalar(
                        out=jt2, in0=it, scalar1=1.0, scalar2=0.0, op0=ALU.mult,
                        op1=ALU.add, accum_out=poi[:, t : t + 1],
                    )

            # ---------------- Phase 1: stream full f2, f3 ----------------
            # b=0 -> f2_full (rhs of pair0), b=1 -> f3_full (rhs of pair1, pair2)
            # Two-pass software pipeline per feature with a lag of LAG tiles:
            #   produce(b, nt): load f32 rows, squared norms, rsqrt, normalize
            #                   to bf16, write to DRAM scratch (row-major)
            #   consume(b, nt): one xbar DMA transpose scratch -> zT tiles,
            #                   then the matmul/exp/colsum block.
            # This keeps the sync DMA FIFO free of long produce->consume chains
            # so the transposes prefetch ahead of the PE stream.
            full_dram = [f2f, f3f]
            partners_of = [[(0, 0)], [(1, 0), (2, 1)]]
            scratch_b = [
                dramp.tile([B, D], bf16, tag=f"scratch{b}", name=f"scratch{b}")
                for b in range(2)
            ]

            def produce(b, nt):
                nsq_nt = smallp.tile([P, ST], f32, tag="small")
                recip = smallp.tile([P, ST], f32, tag="recipnt")
                rf_tiles = []
                for t in range(ST):
                    rf = stage.tile([P, D], f32, tag="rowf32")
                    nc.sync.dma_start(rf, full_dram[b][nt * NTC + t * P : nt * NTC + (t + 1) * P, :])
                    rf_tiles.append(rf)
                    slot = nsq_nt[:, t : t + 1]
                    if t % 2 == 0:
                        jt = junkp.tile([P, D], bf16, tag="junk16")
                        nc.vector.scalar_tensor_tensor(
                            out=jt, in0=rf, scalar=1.0, in1=rf,
                            op0=ALU.mult, op1=ALU.mult, accum_out=slot,
                        )
                    else:
                        jt = junkp.tile([P, D], bf16, tag="junk16")
                        nc.scalar.activation(jt, rf, AF.Square, accum_out=slot)
                rsqrt_act(recip, nsq_nt, ST, tag="ln_nt")
                for t in range(ST):
                    zrow = stage16.tile([P, D], bf16, tag="rowbf16n")
                    nc.vector.tensor_scalar_mul(zrow, rf_tiles[t], recip[:, t : t + 1])
                    nc.gpsimd.dma_start(scratch_b[b][nt * NTC + t * P : nt * NTC + (t + 1) * P, :], zrow)

            def consume(b, nt):
                zT_rhs = rhsp.tile([P, K, NTC], bf16, tag="zTr")
                nc.sync.dma_start_transpose(
                    zT_rhs[:, :, :], scratch_b[b][nt * NTC : (nt + 1) * NTC, :]
                )
                for (pair, a) in partners_of[b]:
                    exp_tiles = []
                    for m in range(MT):
                        ps = ps_s.tile([P, NTC], f32, tag="ps_s")
                        for k in range(K):
                            nc.tensor.matmul(
                                ps,
                                zT_own[a][:, k, ts(m, P)],
                                zT_rhs[:, k, :],
                                start=(k == 0),
                                stop=(k == K - 1),
                            )
                        es = expp.tile([P, NTC], bf16, tag="exps")
                        slot = (pair * MT + m) * NT + nt
                        nc.scalar.activation(
                            es, ps, AF.Exp,
                            scale=scaleA[:, a * MT + m : a * MT + m + 1],
                            accum_out=rowparts[:, slot : slot + 1],
                        )
                        exp_tiles.append(es)
                    cps = ps_c.tile([1, NTC], f32, tag="ps_c")
                    for m in range(MT):
                        nc.tensor.matmul(
                            cps, ones, exp_tiles[m],
                            start=(m == 0), stop=(m == MT - 1),
                        )
                    colp = colpp.tile([1, NTC], f32, tag="colp")
                    nc.any.tensor_copy(out=colp, in_=cps)
                    nc.gpsimd.dma_start(
                        colparts_d[:, pair * B + nt * NTC : pair * B + (nt + 1) * NTC],
                        colp,
                    )

            LAG = 2
            for b in range(2):
                for nt in range(NT + LAG):
                    if nt < NT:
                        produce(b, nt)
                    if nt - LAG >= 0:
                        consume(b, nt - LAG)

            phase0_tail()

            # ---------------- outputs ----------------
            nc.gpsimd.dma_start(rowparts_d, rowparts)
            nc.gpsimd.dma_start(nsq_d, nsq_own)
            nc.gpsimd.dma_start(dots_d, dots_own)
            nc.gpsimd.dma_start(poi_d, poi)

    nc.compile()
    return nc


def make_in_maps(cfg: Cfg, inputs, targets, feature1, feature2, feature3):
    f32 = np.float32
    ac = np.ascontiguousarray
    maps = []
    for c in range(cfg.n_cores):
        sl = slice(c * cfg.S, (c + 1) * cfg.S)
        maps.append({
            "f1_own": ac(feature1[sl], dtype=f32),
            "f2_own": ac(feature2[sl], dtype=f32),
            "f3_own": ac(feature3[sl], dtype=f32),
            "f2_full": ac(feature2, dtype=f32),
            "f3_full": ac(feature3, dtype=f32),
            "inp_own": ac(inputs[sl], dtype=f32),
            "tgt_own": ac(targets[sl], dtype=f32),
        })
    return maps


def combine_results(cfg: Cfg, per_core):
    """per_core: list of dicts with rowparts/colparts/nsq_own/dots_own/poi."""
    B, MT, NT, S = cfg.B, cfg.MT, cfg.NT, cfg.S
    nsq = np.zeros((3, B), np.float64)
    dots = np.zeros((3, B), np.float64)
    rowsum = np.zeros((3, B), np.float64)
    colsum = np.zeros((3, B), np.float64)
    poi_in = 0.0
    poi_tl = 0.0
    for c, r in enumerate(per_core):
        rp = np.asarray(r["rowparts"], np.float64)      # [128, 3*MT*NT]
        cp = np.asarray(r["colparts"], np.float64)[0]   # [3*B]
        nq = np.asarray(r["nsq_own"], np.float64)       # [128, 3*MT]
        dt_ = np.asarray(r["dots_own"], np.float64)
        po = np.asarray(r["poi"], np.float64)           # [128, 2*MT]
        for fi in range(3):
            for t in range(MT):
                nsq[fi, c * S + t * P : c * S + (t + 1) * P] = nq[:, fi * MT + t]
        for pi in range(3):
            for m in range(MT):
                rows = slice(c * S + m * P, c * S + (m + 1) * P)
                dots[pi, rows] = dt_[:, pi * MT + m]
                rowsum[pi, rows] = rp[:, (pi * MT + m) * NT : (pi * MT + m + 1) * NT].sum(axis=1)
            colsum[pi] += cp[pi * B : (pi + 1) * B]
        poi_in += po[:, :MT].sum()
        poi_tl += po[:, MT:].sum()

    na = np.sqrt(nsq)  # [3, B]
    pairs = ((0, 1), (0, 2), (1, 2))
    closs = 0.0
    for pi, (ia, ib) in enumerate(pairs):
        simdiag = dots[pi] / (na[ia] * na[ib])
        loss_i = np.mean(np.log(rowsum[pi]) - simdiag / TEMPERATURE)
        loss_j = np.mean(np.log(colsum[pi]) - simdiag / TEMPERATURE)
        closs += 0.5 * (loss_i + loss_j)
    closs /= 3.0
    p_loss = (poi_in - poi_tl) / (cfg.B * cfg.D)
    total = p_loss + closs
    return (
        np.float32(total),
        np.float32(p_loss),
        np.float32(closs),
    )


_CACHE = {}


def _get_compiled(cfg: Cfg):
    key = (cfg.B, cfg.D, cfg.n_cores, cfg.NTC)
    if key not in _CACHE:
        _CACHE[key] = build_bass(cfg)
    return _CACHE[key]


def kernel(inputs, targets, feature1, feature2, feature3):
    from concourse.bass_utils import run_bass_kernel_spmd

    cfg = Cfg(B=inputs.shape[0], D=inputs.shape[1], n_cores=8, ntc=512)
    nc = _get_compiled(cfg)
    in_maps = make_in_maps(cfg, inputs, targets, feature1, feature2, feature3)
    res = run_bass_kernel_spmd(nc, in_maps, core_ids=list(range(cfg.n_cores)))
    return combine_results(cfg, res.results)


if __name__ == "__main__":
    # smoke test on hardware with full shapes
    rng = np.random.default_rng(0)
    B, D = 4096, 1024
    ins = {
        "inputs": rng.random((B, D), np.float32),
        "targets": rng.random((B, D), np.float32),
        "feature1": rng.standard_normal((B, D), np.float32),
        "feature2": rng.standard_normal((B, D), np.float32),
        "feature3": rng.standard_normal((B, D), np.float32),
    }
    out = kernel(**ins)
    print(out)



# revision 4
# speedup vs baseline: 1.4374x; 1.4374x over previous
"""Trainium2 Bass kernel for nn_CombinedLoss (Poisson + 3-way pairwise CLIP loss).

Strategy (8 NeuronCores, SPMD, one tiny AllGather):
  - Row-shard the batch: core c owns rows [c*512, (c+1)*512) of every tensor.
  - Similarity tiles are computed TRANSPOSED: for pair (a,b) the stationary
    matmul operand is the full feature b, host-transposed to [D, B] and cast
    to fp8/bf16 (a pure layout/dtype change -- all math stays on device); the
    moving operand is this core's own 512 rows of a, normalized on-device and
    PE-transposed.  out[j, i] = z_a[i] . b_raw[j].
  - With j on the PSUM partition axis, the 1/||b_j|| normalization folds into
    the exp's per-partition scale: exp(out * recipB_j / T).  Nobody ever
    normalizes (or even loads a row-major copy of) the full features.
  - recipB for all 4096 rows: each core computes squared norms of its OWN
    rows (DVE square-accum, nearly free) and a 4KB AllGather shares them.
  - Row sums of exp(sim/T): ones-matmul over the j-partition axis, PSUM-
    accumulated across all 32 j-tiles -> complete on device.
  - Column-sum partials: free via the exp activation's accum_out.
  - fp8 (e4m3) DoubleRow matmuls double PE throughput; fp32 accumulate.
  - Host does only the O(B) final combine: log of sums, means.
"""

import sys

import numpy as np

sys.path.insert(0, "/opt/trn_rl_repo")

P = 128
TEMPERATURE = 0.5
EPS_POISSON = 1e-8


class Cfg:
    def __init__(self, B=4096, D=1024, n_cores=8, fp8=True):
        self.B = B          # batch
        self.D = D          # feature dim
        self.n_cores = n_cores
        self.fp8 = fp8
        self.S = B // n_cores      # own rows per core
        self.MT = self.S // P      # own-row tiles (streaming free dim chunks)
        self.K = D // P            # contraction tiles
        self.JT = B // P           # stationary j tiles
        assert B % n_cores == 0 and self.S % P == 0 and D % P == 0
        if fp8:
            assert self.K % 2 == 0


def _patch_act_tables():
    """Make Bacc's act-table pass pick `natural_log_exp_and_others` for both
    Exp and Ln (they otherwise land in two different sets, and alternating
    Ln/Exp calls reload the 2.7us activation tables every tile)."""
    import functools

    import concourse.hw_specs as hw_specs

    if getattr(hw_specs, "_act_tables_patched", False):
        return
    orig = hw_specs.get_activation_tables

    @functools.cache
    def patched(module_arch):
        tabs = dict(orig(module_arch))
        names = list(tabs.keys())
        if "natural_log_exp_and_others" in tabs:
            combined = tabs["natural_log_exp_and_others"]
            for name in names:
                if name == "natural_log_exp_and_others":
                    break
                if tabs[name] & combined:
                    tabs[name] = tabs[name] - combined
        return tabs

    hw_specs.get_activation_tables = patched
    import concourse.bacc as bacc_mod

    if hasattr(bacc_mod, "get_activation_tables"):
        bacc_mod.get_activation_tables = patched
    hw_specs._act_tables_patched = True


def build_bass(cfg: Cfg):
    """Build the single-core Bass program (same program for all SPMD cores)."""
    import concourse.bacc as bacc
    import concourse.bass as bass
    import concourse.mybir as mybir
    import concourse.tile as tile
    from concourse.masks import make_identity

    _patch_act_tables()

    f32 = mybir.dt.float32
    bf16 = mybir.dt.bfloat16
    fp8 = mybir.dt.float8e4
    AF = mybir.ActivationFunctionType
    ALU = mybir.AluOpType
    ts = bass.ts

    B, D, K, MT, JT, S = cfg.B, cfg.D, cfg.K, cfg.MT, cfg.JT, cfg.S
    NC = cfg.n_cores
    mm_dt = fp8 if cfg.fp8 else bf16

    nc = bacc.Bacc(
        "TRN2",
        target_bir_lowering=False,
        debug=False,
        enable_asserts=False,
        num_devices=NC,
    )

    # ---- IO ----
    # fT2/fT3: full features 2,3 transposed [D, B], raw (unnormalized)
    fT2 = nc.dram_tensor("fT2", [D, B], mm_dt, kind="ExternalInput").ap()
    fT3 = nc.dram_tensor("fT3", [D, B], mm_dt, kind="ExternalInput").ap()
    # own row slices, bf16
    f1o = nc.dram_tensor("f1_own", [S, D], bf16, kind="ExternalInput").ap()
    f2o = nc.dram_tensor("f2_own", [S, D], bf16, kind="ExternalInput").ap()
    f3o = nc.dram_tensor("f3_own", [S, D], bf16, kind="ExternalInput").ap()
    inp = nc.dram_tensor("inp_own", [S, D], bf16, kind="ExternalInput").ap()
    tgt = nc.dram_tensor("tgt_own", [S, D], bf16, kind="ExternalInput").ap()

    rowsum_d = nc.dram_tensor("rowsum", [1, 3 * S], f32, kind="ExternalOutput").ap()
    colparts_d = nc.dram_tensor("colparts", [P, 3 * JT], f32, kind="ExternalOutput").ap()
    nsq_d = nc.dram_tensor("nsq_own", [P, 3 * MT], f32, kind="ExternalOutput").ap()
    dots_d = nc.dram_tensor("dots_own", [P, 3 * MT], f32, kind="ExternalOutput").ap()
    poi_d = nc.dram_tensor("poi", [P, 2 * MT], f32, kind="ExternalOutput").ap()

    own_dram = [f1o, f2o, f3o]
    fT_dram = [fT2, fT3]

    with tile.TileContext(nc) as tc:
        with (
            tc.tile_pool(name="const", bufs=1) as const_pool,
            tc.tile_pool(name="persist", bufs=1) as persist,
            tc.tile_pool(name="own", bufs=1) as ownp,
            tc.tile_pool(name="stage", bufs=4) as stage,
            tc.tile_pool(name="junk", bufs=2) as junkp,
            tc.tile_pool(name="exps", bufs=4) as expp,
            tc.tile_pool(name="small", bufs=8) as smallp,
            tc.tile_pool(name="dscr", bufs=1, space="DRAM") as dramp,
            tc.tile_pool(name="ps_s", bufs=4, space="PSUM") as ps_s,
            tc.tile_pool(name="ps_t", bufs=2, space="PSUM") as ps_t,
            tc.tile_pool(name="ps_r", bufs=2, space="PSUM") as ps_r,
        ):
            identity = const_pool.tile([P, P], bf16)
            make_identity(nc, identity)
            ones = const_pool.tile([P, 1], bf16)
            nc.vector.memset(ones, 1.0)
            eps_bias = const_pool.tile([P, 1], f32)
            nc.vector.memset(eps_bias, EPS_POISSON)

            # persistent state
            fT_sb = [persist.tile([P, K, B], mm_dt, name=f"fT_sb{i}") for i in range(2)]
            zT = [persist.tile([P, K, S], mm_dt, name=f"zT{i}") for i in range(2)]
            nsq_own = persist.tile([P, 3 * MT], f32)
            dots_own = persist.tile([P, 3 * MT], f32)
            poi = persist.tile([P, 2 * MT], f32)
            colparts = persist.tile([P, 3 * JT], f32)
            rowsum_sb = persist.tile([1, 3 * S], f32)
            scaleB = [persist.tile([P, JT], f32, name=f"scaleB{i}") for i in range(2)]

            own_rf = {}  # (fi, t) -> own bf16 row tile

            def rsqrt_act(dst, src, n, tag):
                # dst[:, :n] = 1/sqrt(src) = exp(-0.5*ln(src))
                l = smallp.tile([P, n], f32, tag=tag)
                nc.scalar.activation(l, src, AF.Ln)
                nc.scalar.activation(dst, l, AF.Exp, scale=-0.5)

            # ---- Phase A1: own f2/f3 squared norms -> collective, ASAP ----
            for fi in (1, 2):
                for t in range(MT):
                    rf = ownp.tile([P, D], bf16, tag=f"own{fi}_{t}")
                    nc.sync.dma_start(rf, own_dram[fi][ts(t, P), :])
                    own_rf[(fi, t)] = rf
                    jt_ = junkp.tile([P, D], bf16, tag="junk16")
                    nc.vector.scalar_tensor_tensor(
                        out=jt_, in0=rf, scalar=1.0, in1=rf,
                        op0=ALU.mult, op1=ALU.mult,
                        accum_out=nsq_own[:, fi * MT + t : fi * MT + t + 1],
                    )
            cc_in = dramp.tile([P, 2 * MT], f32, name="cc_in")
            cc_out = dramp.tile([NC * P, 2 * MT], f32, name="cc_out")
            nc.gpsimd.dma_start(cc_in[:], nsq_own[:, MT : 3 * MT])
            nc.gpsimd.collective_compute(
                "AllGather", mybir.AluOpType.bypass,
                replica_groups=[list(range(NC))],
                ins=[cc_in.opt()], outs=[cc_out.opt()],
            )

            # ---- Phase A2: stationary fT loads (big, overlap everything) ----
            for b in range(2):
                for k in range(K):
                    nc.sync.dma_start(fT_sb[b][:, k, :], fT_dram[b][ts(k, P), :])

            # ---- Phase A3: own f1, diagonal dots, own normalize+transpose ----
            for t in range(MT):
                rf = ownp.tile([P, D], bf16, tag=f"own0_{t}")
                nc.sync.dma_start(rf, own_dram[0][ts(t, P), :])
                own_rf[(0, t)] = rf
                jt_ = junkp.tile([P, D], bf16, tag="junk16")
                nc.vector.scalar_tensor_tensor(
                    out=jt_, in0=rf, scalar=1.0, in1=rf,
                    op0=ALU.mult, op1=ALU.mult,
                    accum_out=nsq_own[:, t : t + 1],
                )
            for pi, (ia, ib) in enumerate(((0, 1), (0, 2), (1, 2))):
                for t in range(MT):
                    jt_ = junkp.tile([P, D], bf16, tag="junk16")
                    nc.vector.scalar_tensor_tensor(
                        out=jt_, in0=own_rf[(ia, t)], scalar=1.0, in1=own_rf[(ib, t)],
                        op0=ALU.mult, op1=ALU.mult,
                        accum_out=dots_own[:, pi * MT + t : pi * MT + t + 1],
                    )

            recip_own = smallp.tile([P, 2 * MT], f32, tag="recip_own")
            rsqrt_act(recip_own, nsq_own[:, : 2 * MT], 2 * MT, tag="ln_own")

            # normalize own f1/f2 rows, PE-transpose into zT (cast to mm_dt)
            for a in range(2):
                for t in range(MT):
                    zrow = stage.tile([P, D], bf16, tag="zrow")
                    nc.vector.tensor_scalar_mul(
                        zrow, own_rf[(a, t)], recip_own[:, a * MT + t : a * MT + t + 1]
                    )
                    tps = ps_t.tile([P, K * P], bf16, tag="tps")
                    for k in range(K):
                        nc.tensor.transpose(tps[:, ts(k, P)], zrow[:, ts(k, P)], identity)
                    nc.any.tensor_copy(
                        out=zT[a][:, :, ts(t, P)],
                        in_=tps[:].rearrange("p (k c) -> p k c", k=K),
                    )

            # ---- Phase A4: gathered norms -> per-partition exp scales ----
            # cc_out rows r*P..(r+1)*P hold core r's [128, 2*MT] slab; global
            # j-tile r*MT+m lives at rows [r*P:(r+1)*P], cols fi*MT+m.
            for b in range(2):
                nsqB = smallp.tile([P, JT], f32, tag=f"nsqB{b}")
                for r in range(NC):
                    nc.scalar.dma_start(
                        nsqB[:, r * MT : (r + 1) * MT],
                        cc_out[r * P : (r + 1) * P, b * MT : (b + 1) * MT],
                    )
                recipB = smallp.tile([P, JT], f32, tag=f"recipB{b}")
                rsqrt_act(recipB, nsqB, JT, tag=f"lnB{b}")
                nc.vector.tensor_scalar_mul(scaleB[b], recipB, 1.0 / TEMPERATURE)

            # ---- Phase B: main matmuls ----
            # pair -> (a stream idx into zT, b stationary idx into fT_sb)
            pairs = ((0, 0), (0, 1), (1, 1))  # (f1,f2), (f1,f3), (f2,f3)

            def mains(pi, a, b, jt):
                ps = ps_s.tile([P, S], f32, tag="ps_s")
                if cfg.fp8:
                    for kk in range(0, K, 2):
                        nc.tensor.matmul(
                            ps,
                            fT_sb[b][:, kk : kk + 2, ts(jt, P)],
                            zT[a][:, kk : kk + 2, :],
                            start=(kk == 0),
                            stop=(kk == K - 2),
                            perf_mode=mybir.MatmulPerfMode.DoubleRow,
                        )
                else:
                    for k in range(K):
                        nc.tensor.matmul(
                            ps,
                            fT_sb[b][:, k, ts(jt, P)],
                            zT[a][:, k, :],
                            start=(k == 0),
                            stop=(k == K - 1),
                        )
                es = expp.tile([P, S], bf16, tag="exps")
                slot = pi * JT + jt
                nc.scalar.activation(
                    es, ps, AF.Exp,
                    scale=scaleB[b][:, jt : jt + 1],
                    accum_out=colparts[:, slot : slot + 1],
                )
                return es

            for pi, (a, b) in enumerate(pairs):
                rs_ps = ps_r.tile([1, S], f32, tag="ps_r")
                es_prev = None
                for jt in range(JT + 1):
                    if jt < JT:
                        es = mains(pi, a, b, jt)
                    if jt >= 1:
                        nc.tensor.matmul(
                            rs_ps, ones, es_prev,
                            start=(jt == 1), stop=(jt == JT),
                        )
                    if jt < JT:
                        es_prev = es
                nc.any.tensor_copy(out=rowsum_sb[:, pi * S : (pi + 1) * S], in_=rs_ps)

            # ---- Phase C: poisson tail ----
            for t in range(MT):
                it = stage.tile([P, D], bf16, tag="it")
                tt = stage.tile([P, D], bf16, tag="tt")
                nc.sync.dma_start(it, inp[ts(t, P), :])
                nc.sync.dma_start(tt, tgt[ts(t, P), :])
                lg = stage.tile([P, D], f32, tag="lg")
                nc.scalar.activation(lg, it, AF.Ln, bias=eps_bias[:, :])
                jt_ = junkp.tile([P, D], bf16, tag="junk16")
                nc.vector.scalar_tensor_tensor(
                    out=jt_, in0=tt, scalar=1.0, in1=lg,
                    op0=ALU.mult, op1=ALU.mult,
                    accum_out=poi[:, MT + t : MT + t + 1],
                )
                jt2 = junkp.tile([P, D], bf16, tag="junk16")
                nc.vector.tensor_scalar(
                    out=jt2, in0=it, scalar1=1.0, scalar2=0.0, op0=ALU.mult,
                    op1=ALU.add, accum_out=poi[:, t : t + 1],
                )

            # ---- outputs ----
            nc.gpsimd.dma_start(rowsum_d, rowsum_sb)
            nc.gpsimd.dma_start(colparts_d, colparts)
            nc.gpsimd.dma_start(nsq_d, nsq_own)
            nc.gpsimd.dma_start(dots_d, dots_own)
            nc.gpsimd.dma_start(poi_d, poi)

    nc.compile()
    return nc


def make_in_maps(cfg: Cfg, inputs, targets, feature1, feature2, feature3):
    import ml_dtypes

    bf16 = ml_dtypes.bfloat16
    mm_np = ml_dtypes.float8_e4m3 if cfg.fp8 else bf16
    ac = np.ascontiguousarray

    # shared across cores: full transposed raw features (layout+dtype only)
    fT2 = ac(feature2.T).astype(mm_np)
    fT3 = ac(feature3.T).astype(mm_np)
    f1b = feature1.astype(bf16)
    f2b = feature2.astype(bf16)
    f3b = feature3.astype(bf16)
    inb = inputs.astype(bf16)
    tgb = targets.astype(bf16)

    maps = []
    for c in range(cfg.n_cores):
        sl = slice(c * cfg.S, (c + 1) * cfg.S)
        maps.append({
            "fT2": fT2,
            "fT3": fT3,
            "f1_own": ac(f1b[sl]),
            "f2_own": ac(f2b[sl]),
            "f3_own": ac(f3b[sl]),
            "inp_own": ac(inb[sl]),
            "tgt_own": ac(tgb[sl]),
        })
    return maps


def combine_results(cfg: Cfg, per_core):
    """per_core: list of dicts with rowsum/colparts/nsq_own/dots_own/poi."""
    B, MT, JT, S = cfg.B, cfg.MT, cfg.JT, cfg.S
    nsq = np.zeros((3, B), np.float64)
    dots = np.zeros((3, B), np.float64)
    rowsum = np.zeros((3, B), np.float64)
    colsum = np.zeros((3, B), np.float64)
    poi_in = 0.0
    poi_tl = 0.0
    for c, r in enumerate(per_core):
        rs = np.asarray(r["rowsum"], np.float64)[0]    # [3*S]
        cp = np.asarray(r["colparts"], np.float64)     # [128, 3*JT]
        nq = np.asarray(r["nsq_own"], np.float64)      # [128, 3*MT]
        dt_ = np.asarray(r["dots_own"], np.float64)
        po = np.asarray(r["poi"], np.float64)          # [128, 2*MT]
        for fi in range(3):
            for t in range(MT):
                nsq[fi, c * S + t * P : c * S + (t + 1) * P] = nq[:, fi * MT + t]
        for pi in range(3):
            rowsum[pi, c * S : (c + 1) * S] = rs[pi * S : (pi + 1) * S]
            for t in range(MT):
                rows = slice(c * S + t * P, c * S + (t + 1) * P)
                dots[pi, rows] = dt_[:, pi * MT + t]
            for jt in range(JT):
                colsum[pi, jt * P : (jt + 1) * P] += cp[:, pi * JT + jt]
        poi_in += po[:, :MT].sum()
        poi_tl += po[:, MT:].sum()

    na = np.sqrt(nsq)  # [3, B]
    pairs = ((0, 1), (0, 2), (1, 2))
    closs = 0.0
    for pi, (ia, ib) in enumerate(pairs):
        simdiag = dots[pi] / (na[ia] * na[ib])
        loss_i = np.mean(np.log(rowsum[pi]) - simdiag / TEMPERATURE)
        loss_j = np.mean(np.log(colsum[pi]) - simdiag / TEMPERATURE)
        closs += 0.5 * (loss_i + loss_j)
    closs /= 3.0
    p_loss = (poi_in - poi_tl) / (cfg.B * cfg.D)
    total = p_loss + closs
    return (
        np.float32(total),
        np.float32(p_loss),
        np.float32(closs),
    )


_CACHE = {}


def _get_compiled(cfg: Cfg):
    key = (cfg.B, cfg.D, cfg.n_cores, cfg.fp8)
    if key not in _CACHE:
        _CACHE[key] = build_bass(cfg)
    return _CACHE[key]


def kernel(inputs, targets, feature1, feature2, feature3):
    from concourse.bass_utils import run_bass_kernel_spmd

    cfg = Cfg(B=inputs.shape[0], D=inputs.shape[1], n_cores=8)
    nc = _get_compiled(cfg)
    in_maps = make_in_maps(cfg, inputs, targets, feature1, feature2, feature3)
    res = run_bass_kernel_spmd(nc, in_maps, core_ids=list(range(cfg.n_cores)))
    return combine_results(cfg, res.results)


if __name__ == "__main__":
    rng = np.random.default_rng(0)
    B, D = 4096, 1024
    ins = {
        "inputs": rng.random((B, D), np.float32),
        "targets": rng.random((B, D), np.float32),
        "feature1": rng.standard_normal((B, D)).astype(np.float32),
        "feature2": rng.standard_normal((B, D)).astype(np.float32),
        "feature3": rng.standard_normal((B, D)).astype(np.float32),
    }
    out = kernel(**ins)
    print(out)


# revision 14
# speedup vs baseline: 1.9099x; 1.3287x over previous
"""Trainium2 Bass kernel for nn_CombinedLoss (Poisson + 3-way pairwise CLIP loss).

Strategy (8 NeuronCores, SPMD, one tiny AllGather):
  - Row-shard the batch: core c owns rows [c*512, (c+1)*512) of every tensor.
  - Similarity tiles are computed TRANSPOSED: for pair (a,b) the stationary
    matmul operand is the full feature b, host-transposed to [D, B] and cast
    to fp8/bf16 (a pure layout/dtype change -- all math stays on device); the
    moving operand is this core's own 512 rows of a, normalized on-device and
    PE-transposed.  out[j, i] = z_a[i] . b_raw[j].
  - With j on the PSUM partition axis, the 1/||b_j|| normalization folds into
    the exp's per-partition scale: exp(out * recipB_j / T).  Nobody ever
    normalizes (or even loads a row-major copy of) the full features.
  - recipB for all 4096 rows: each core computes squared norms of its OWN
    rows (DVE square-accum, nearly free) and a 4KB AllGather shares them.
  - Row sums of exp(sim/T): ones-matmul over the j-partition axis, PSUM-
    accumulated across all 32 j-tiles -> complete on device.
  - Column-sum partials: free via the exp activation's accum_out.
  - fp8 (e4m3) DoubleRow matmuls double PE throughput; fp32 accumulate.
  - Host does only the O(B) final combine: log of sums, means.
"""

import sys

import numpy as np

sys.path.insert(0, "/opt/trn_rl_repo")

P = 128
TEMPERATURE = 0.5
EPS_POISSON = 1e-8


class Cfg:
    def __init__(self, B=4096, D=1024, n_cores=8, fp8=True):
        self.B = B          # batch
        self.D = D          # feature dim
        self.n_cores = n_cores
        self.fp8 = fp8
        self.S = B // n_cores      # own rows per core
        self.MT = self.S // P      # own-row tiles (streaming free dim chunks)
        self.K = D // P            # contraction tiles
        self.JT = B // P           # stationary j tiles
        assert B % n_cores == 0 and self.S % P == 0 and D % P == 0
        if fp8:
            assert self.K % 2 == 0


def _patch_act_tables():
    """Make Bacc's act-table pass pick `natural_log_exp_and_others` for both
    Exp and Ln (they otherwise land in two different sets, and alternating
    Ln/Exp calls reload the 2.7us activation tables every tile)."""
    import functools

    import concourse.hw_specs as hw_specs

    if getattr(hw_specs, "_act_tables_patched", False):
        return
    orig = hw_specs.get_activation_tables

    @functools.cache
    def patched(module_arch):
        tabs = dict(orig(module_arch))
        names = list(tabs.keys())
        if "natural_log_exp_and_others" in tabs:
            combined = tabs["natural_log_exp_and_others"]
            for name in names:
                if name == "natural_log_exp_and_others":
                    break
                if tabs[name] & combined:
                    tabs[name] = tabs[name] - combined
        return tabs

    hw_specs.get_activation_tables = patched
    import concourse.bacc as bacc_mod

    if hasattr(bacc_mod, "get_activation_tables"):
        bacc_mod.get_activation_tables = patched
    hw_specs._act_tables_patched = True


def build_bass(cfg: Cfg):
    """Build the single-core Bass program (same program for all SPMD cores)."""
    import concourse.bacc as bacc
    import concourse.bass as bass
    import concourse.mybir as mybir
    import concourse.tile as tile
    from concourse.masks import make_identity

    _patch_act_tables()

    f32 = mybir.dt.float32
    bf16 = mybir.dt.bfloat16
    fp8 = mybir.dt.float8e4
    AF = mybir.ActivationFunctionType
    ALU = mybir.AluOpType
    ts = bass.ts

    B, D, K, MT, JT, S = cfg.B, cfg.D, cfg.K, cfg.MT, cfg.JT, cfg.S
    NC = cfg.n_cores
    mm_dt = fp8 if cfg.fp8 else bf16

    nc = bacc.Bacc(
        "TRN2",
        target_bir_lowering=False,
        debug=False,
        enable_asserts=False,
        num_devices=NC,
    )

    # ---- IO ----
    # fT2/fT3: full features 2,3 transposed [D, B], raw (unnormalized)
    fT2 = nc.dram_tensor("fT2", [D, B], mm_dt, kind="ExternalInput").ap()
    fT3 = nc.dram_tensor("fT3", [D, B], mm_dt, kind="ExternalInput").ap()
    # own row slices, bf16
    f1o = nc.dram_tensor("f1_own", [S, D], bf16, kind="ExternalInput").ap()
    f2o = nc.dram_tensor("f2_own", [S, D], bf16, kind="ExternalInput").ap()
    f3o = nc.dram_tensor("f3_own", [S, D], bf16, kind="ExternalInput").ap()
    inp = nc.dram_tensor("inp_own", [S, D], bf16, kind="ExternalInput").ap()
    tgt = nc.dram_tensor("tgt_own", [S, D], bf16, kind="ExternalInput").ap()

    rowsum_d = nc.dram_tensor("rowsum", [1, 3 * S], f32, kind="ExternalOutput").ap()
    colparts_d = nc.dram_tensor("colparts", [P, 3 * JT], f32, kind="ExternalOutput").ap()
    nsq_d = nc.dram_tensor("nsq_own", [P, 3 * MT], f32, kind="ExternalOutput").ap()
    dots_d = nc.dram_tensor("dots_own", [P, 3 * MT], f32, kind="ExternalOutput").ap()
    poi_d = nc.dram_tensor("poi", [P, 2 * MT], f32, kind="ExternalOutput").ap()

    own_dram = [f1o, f2o, f3o]
    fT_dram = [fT2, fT3]

    with tile.TileContext(nc) as tc:
        with (
            tc.tile_pool(name="const", bufs=1) as const_pool,
            tc.tile_pool(name="persist", bufs=1) as persist,
            tc.tile_pool(name="own", bufs=1) as ownp,
            tc.tile_pool(name="stage", bufs=2) as stage,
            tc.tile_pool(name="junk", bufs=2) as junkp,
            tc.tile_pool(name="exps", bufs=6) as expp,
            tc.tile_pool(name="small", bufs=8) as smallp,
            tc.tile_pool(name="dscr", bufs=1, space="DRAM") as dramp,
            tc.tile_pool(name="ps_s", bufs=4, space="PSUM") as ps_s,
            tc.tile_pool(name="ps_t", bufs=1, space="PSUM") as ps_t,
            tc.tile_pool(name="ps_r", bufs=3, space="PSUM") as ps_r,
        ):
            identity = const_pool.tile([P, P], bf16)
            make_identity(nc, identity)
            ones = const_pool.tile([P, 1], bf16)
            nc.vector.memset(ones, 1.0)
            eps_bias = const_pool.tile([P, 1], f32)
            nc.vector.memset(eps_bias, EPS_POISSON)

            # persistent state
            fT_sb = [persist.tile([P, K, B], mm_dt, name=f"fT_sb{i}") for i in range(2)]
            zT = [persist.tile([P, K, S], mm_dt, name=f"zT{i}") for i in range(2)]
            nsq_own = persist.tile([P, 3 * MT], f32)
            dots_own = persist.tile([P, 3 * MT], f32)
            poi = persist.tile([P, 2 * MT], f32)
            colparts = persist.tile([P, 3 * JT], f32)
            rowsum_sb = persist.tile([1, 3 * S], f32)
            scaleB = [persist.tile([P, JT], f32, name=f"scaleB{i}") for i in range(2)]
            # sim-stash for tiles computed before the gathered norms arrive
            N_STASH_F = min(12, JT)   # fused groups stashed (2 tiles each)
            stash12 = persist.tile([P, JT, S], bf16)
            stashF = persist.tile([P, 2 * N_STASH_F, S], bf16)

            own_rf = {}  # (fi, t) -> own bf16 row tile

            def rsqrt_act(dst, src, n, tag):
                # dst[:, :n] = 1/sqrt(src) = exp(-0.5*ln(src))
                l = smallp.tile([P, n], f32, tag=tag)
                nc.scalar.activation(l, src, AF.Ln)
                nc.scalar.activation(dst, l, AF.Exp, scale=-0.5)

            # ---- Phase A1: own loads + squared norms -> collective, ASAP ----
            # sync queue: f2o, f3o, f1o, then fT2 (stationary of pair12)
            for fi in (1, 2, 0):
                for t in range(MT):
                    rf = ownp.tile([P, D], bf16, tag=f"own{fi}_{t}")
                    nc.sync.dma_start(rf, own_dram[fi][ts(t, P), :])
                    own_rf[(fi, t)] = rf
            for t in range(MT):  # f2 squares on DVE
                jt_ = junkp.tile([P, D], bf16, tag="junk16")
                nc.vector.scalar_tensor_tensor(
                    out=jt_, in0=own_rf[(1, t)], scalar=1.0, in1=own_rf[(1, t)],
                    op0=ALU.mult, op1=ALU.mult,
                    accum_out=nsq_own[:, MT + t : MT + t + 1],
                )
            for t in range(MT):  # f3 squares on ACT
                jt_ = junkp.tile([P, D], bf16, tag="junk16")
                nc.scalar.activation(
                    jt_, own_rf[(2, t)], AF.Square,
                    accum_out=nsq_own[:, 2 * MT + t : 2 * MT + t + 1],
                )
            cc_in = dramp.tile([P, 2 * MT], f32, name="cc_in")
            cc_out = dramp.tile([NC * P, 2 * MT], f32, name="cc_out")
            nc.gpsimd.dma_start(cc_in[:], nsq_own[:, MT : 3 * MT])
            nc.gpsimd.collective_compute(
                "AllGather", mybir.AluOpType.bypass,
                replica_groups=[list(range(NC))],
                ins=[cc_in.opt()], outs=[cc_out.opt()],
            )
            for k in range(K):  # fT2 on sync queue
                nc.sync.dma_start(fT_sb[0][:, k, :], fT_dram[0][ts(k, P), :])
            for k in range(K):  # fT3 on gpsimd queue
                nc.gpsimd.dma_start(fT_sb[1][:, k, :], fT_dram[1][ts(k, P), :])

            # ---- Phase A2: f1 norms, diagonal dots, own normalize+transpose ----
            for t in range(MT):
                jt_ = junkp.tile([P, D], bf16, tag="junk16")
                nc.vector.scalar_tensor_tensor(
                    out=jt_, in0=own_rf[(0, t)], scalar=1.0, in1=own_rf[(0, t)],
                    op0=ALU.mult, op1=ALU.mult,
                    accum_out=nsq_own[:, t : t + 1],
                )
            recip_own = smallp.tile([P, 2 * MT], f32, tag="recip_own")
            rsqrt_act(recip_own, nsq_own[:, : 2 * MT], 2 * MT, tag="ln_own")

            # normalize own f1/f2 rows, PE-transpose into zT (cast to mm_dt)
            for a in range(2):
                for t in range(MT):
                    zrow = stage.tile([P, D], bf16, tag="zrow")
                    nc.vector.tensor_scalar_mul(
                        zrow, own_rf[(a, t)], recip_own[:, a * MT + t : a * MT + t + 1]
                    )
                    tps = ps_t.tile([P, K * P], bf16, tag="tps")
                    for k in range(K):
                        nc.tensor.transpose(tps[:, ts(k, P)], zrow[:, ts(k, P)], identity)
                    nc.any.tensor_copy(
                        out=zT[a][:, :, ts(t, P)],
                        in_=tps[:].rearrange("p (k c) -> p k c", k=K),
                    )

            for pi, (ia, ib) in enumerate(((0, 1), (0, 2), (1, 2))):
                for t in range(MT):
                    jt_ = junkp.tile([P, D], bf16, tag="junk16")
                    nc.vector.scalar_tensor_tensor(
                        out=jt_, in0=own_rf[(ia, t)], scalar=1.0, in1=own_rf[(ib, t)],
                        op0=ALU.mult, op1=ALU.mult,
                        accum_out=dots_own[:, pi * MT + t : pi * MT + t + 1],
                    )

            # ---- Phase B: main matmuls ----
            # orientation: out[j, i] = fT_b[:, j] . zT_a[:, i]; stationary fT_b.
            def mm_group(b, a, ps, jt):
                if cfg.fp8:
                    for kk in range(0, K, 2):
                        nc.tensor.matmul(
                            ps,
                            fT_sb[b][:, kk : kk + 2, ts(jt, P)],
                            zT[a][:, kk : kk + 2, :],
                            start=(kk == 0), stop=(kk == K - 2),
                            perf_mode=mybir.MatmulPerfMode.DoubleRow,
                            skip_group_check=True,
                        )
                else:
                    for k in range(K):
                        nc.tensor.matmul(
                            ps,
                            fT_sb[b][:, k, ts(jt, P)],
                            zT[a][:, k, :],
                            start=(k == 0), stop=(k == K - 1),
                            skip_group_check=True,
                        )

            def mm_group_fused(ps13, ps23, jt):
                # pairs (f1,f3) and (f2,f3) share stationary fT3 -> one
                # LDWEIGHTS per (jt, kk) serves two matmuls.
                if cfg.fp8:
                    for kk in range(0, K, 2):
                        w = fT_sb[1][:, kk : kk + 2, ts(jt, P)]
                        for ps, a in ((ps13, 0), (ps23, 1)):
                            nc.tensor.matmul(
                                ps, w, zT[a][:, kk : kk + 2, :],
                                start=(kk == 0), stop=(kk == K - 2),
                                perf_mode=mybir.MatmulPerfMode.DoubleRow,
                                skip_group_check=True,
                            )
                else:
                    for k in range(K):
                        w = fT_sb[1][:, k, ts(jt, P)]
                        for ps, a in ((ps13, 0), (ps23, 1)):
                            nc.tensor.matmul(
                                ps, w, zT[a][:, k, :],
                                start=(k == 0), stop=(k == K - 1),
                                skip_group_check=True,
                            )

            def do_exp(pi, b, jt, src):
                # es = exp(src * recipB_j / T); column-sum partial via DVE
                es = expp.tile([P, S], bf16, tag="exps")
                nc.scalar.activation(
                    es, src, AF.Exp, scale=scaleB[b][:, jt : jt + 1]
                )
                jt_ = junkp.tile([P, S], bf16, tag="junkS")
                slot = pi * JT + jt
                nc.vector.tensor_scalar(
                    out=jt_, in0=es, scalar1=1.0, scalar2=0.0,
                    op0=ALU.mult, op1=ALU.add,
                    accum_out=colparts[:, slot : slot + 1],
                )
                return es

            es_tiles = {}  # (pi, jt) -> es tile

            def build_scaleB():
                # gathered norms -> per-partition exp scales (collective
                # result); emitted mid-M2 so the in-order ACT stream does
                # not block earlier stash copies on the collective.
                nsqB = smallp.tile([P, NC, 2 * MT], f32, tag="nsqB")
                nc.sync.dma_start(
                    nsqB, cc_out[:].rearrange("(r p) m -> p r m", r=NC)
                )
                for b in range(2):
                    lnB = smallp.tile([P, JT], f32, tag=f"lnB{b}")
                    nc.scalar.activation(
                        lnB[:].rearrange("p (r m) -> p r m", r=NC),
                        nsqB[:, :, b * MT : (b + 1) * MT],
                        AF.Ln,
                    )
                    recipB = smallp.tile([P, JT], f32, tag=f"recipB{b}")
                    nc.scalar.activation(recipB, lnB, AF.Exp, scale=-0.5)
                    nc.vector.tensor_scalar_mul(
                        scaleB[b], recipB, 1.0 / TEMPERATURE
                    )

            # M1: pair (f1,f2): all 32 groups stashed (gathered norms not
            # ready yet); copies run on otherwise-idle ACT.
            for jt in range(JT):
                ps = ps_s.tile([P, S], f32, tag="ps_s")
                mm_group(0, 0, ps, jt)
                nc.scalar.activation(stash12[:, jt, :], ps, AF.Copy)

            # M2: fused pairs (f1,f3), (f2,f3): first N_STASH_F groups
            # stashed, rest exp'd directly from PSUM.  Row-sum matmuls for
            # the direct groups interleave with a small lag.
            rs13 = ps_r.tile([1, S], f32, tag="ps_r", name="rs13")
            rs23 = ps_r.tile([1, S], f32, tag="ps_r", name="rs23")
            rsF_started = False
            RLAG = 2
            for jt in range(JT):
                ps13 = ps_s.tile([P, S], f32, tag="ps_s")
                ps23 = ps_s.tile([P, S], f32, tag="ps_s")
                mm_group_fused(ps13, ps23, jt)
                g = jt - RLAG
                if g >= N_STASH_F:
                    nc.tensor.matmul(rs13, ones, es_tiles[(1, g)],
                                     start=(g == N_STASH_F), stop=False,
                                     skip_group_check=True)
                    nc.tensor.matmul(rs23, ones, es_tiles[(2, g)],
                                     start=(g == N_STASH_F), stop=False,
                                     skip_group_check=True)
                    rsF_started = True
                if jt < N_STASH_F:
                    nc.scalar.activation(stashF[:, 2 * jt, :], ps13, AF.Copy)
                    nc.scalar.activation(stashF[:, 2 * jt + 1, :], ps23, AF.Copy)
                else:
                    if jt == N_STASH_F:
                        build_scaleB()
                    es_tiles[(1, jt)] = do_exp(1, 1, jt, ps13)
                    es_tiles[(2, jt)] = do_exp(2, 1, jt, ps23)
            if N_STASH_F >= JT:
                build_scaleB()

            # E1: exp the stashed tiles (wait on scaleB only)
            for jt in range(JT):
                es_tiles[(0, jt)] = do_exp(0, 0, jt, stash12[:, jt, :])
            for jt in range(N_STASH_F):
                es_tiles[(1, jt)] = do_exp(1, 1, jt, stashF[:, 2 * jt, :])
                es_tiles[(2, jt)] = do_exp(2, 1, jt, stashF[:, 2 * jt + 1, :])

            # M3: remaining row-sum matmuls
            #  - trailing direct groups not covered by the M2 lag
            for g in range(max(N_STASH_F, JT - RLAG), JT):
                nc.tensor.matmul(rs13, ones, es_tiles[(1, g)],
                                 start=False, stop=False, skip_group_check=True)
                nc.tensor.matmul(rs23, ones, es_tiles[(2, g)],
                                 start=False, stop=False, skip_group_check=True)
            #  - pair12 (all stashed)
            rs12 = ps_r.tile([1, S], f32, tag="ps_r", name="rs12")
            for jt in range(JT):
                nc.tensor.matmul(rs12, ones, es_tiles[(0, jt)],
                                 start=(jt == 0), stop=(jt == JT - 1),
                                 skip_group_check=True)
            nc.any.tensor_copy(out=rowsum_sb[:, 0:S], in_=rs12)
            #  - stashed fused groups, closing the rs13/rs23 accumulations
            for jt in range(min(N_STASH_F, JT)):
                st = (not rsF_started) and jt == 0
                nc.tensor.matmul(rs13, ones, es_tiles[(1, jt)],
                                 start=st, stop=(jt == min(N_STASH_F, JT) - 1),
                                 skip_group_check=True)
                nc.tensor.matmul(rs23, ones, es_tiles[(2, jt)],
                                 start=st, stop=(jt == min(N_STASH_F, JT) - 1),
                                 skip_group_check=True)
            nc.any.tensor_copy(out=rowsum_sb[:, S : 2 * S], in_=rs13)
            nc.any.tensor_copy(out=rowsum_sb[:, 2 * S : 3 * S], in_=rs23)

            # ---- Phase C: poisson tail ----
            for t in range(MT):
                it = stage.tile([P, D], bf16, tag="it")
                tt = stage.tile([P, D], bf16, tag="tt")
                nc.sync.dma_start(it, inp[ts(t, P), :])
                nc.sync.dma_start(tt, tgt[ts(t, P), :])
                lg = stage.tile([P, D], f32, tag="lg")
                nc.scalar.activation(lg, it, AF.Ln, bias=eps_bias[:, :])
                jt_ = junkp.tile([P, D], bf16, tag="junk16")
                nc.vector.scalar_tensor_tensor(
                    out=jt_, in0=tt, scalar=1.0, in1=lg,
                    op0=ALU.mult, op1=ALU.mult,
                    accum_out=poi[:, MT + t : MT + t + 1],
                )
                jt2 = junkp.tile([P, D], bf16, tag="junk16")
                nc.vector.tensor_scalar(
                    out=jt2, in0=it, scalar1=1.0, scalar2=0.0, op0=ALU.mult,
                    op1=ALU.add, accum_out=poi[:, t : t + 1],
                )

            # ---- outputs ----
            nc.gpsimd.dma_start(rowsum_d, rowsum_sb)
            nc.gpsimd.dma_start(colparts_d, colparts)
            nc.gpsimd.dma_start(nsq_d, nsq_own)
            nc.gpsimd.dma_start(dots_d, dots_own)
            nc.gpsimd.dma_start(poi_d, poi)

    nc.compile()
    return nc


def make_in_maps(cfg: Cfg, inputs, targets, feature1, feature2, feature3):
    import ml_dtypes

    bf16 = ml_dtypes.bfloat16
    mm_np = ml_dtypes.float8_e4m3 if cfg.fp8 else bf16
    ac = np.ascontiguousarray

    # shared across cores: full transposed raw features (layout+dtype only)
    fT2 = ac(feature2.T).astype(mm_np)
    fT3 = ac(feature3.T).astype(mm_np)
    f1b = feature1.astype(bf16)
    f2b = feature2.astype(bf16)
    f3b = feature3.astype(bf16)
    inb = inputs.astype(bf16)
    tgb = targets.astype(bf16)

    maps = []
    for c in range(cfg.n_cores):
        sl = slice(c * cfg.S, (c + 1) * cfg.S)
        maps.append({
            "fT2": fT2,
            "fT3": fT3,
            "f1_own": ac(f1b[sl]),
            "f2_own": ac(f2b[sl]),
            "f3_own": ac(f3b[sl]),
            "inp_own": ac(inb[sl]),
            "tgt_own": ac(tgb[sl]),
        })
    return maps


def combine_results(cfg: Cfg, per_core):
    """per_core: list of dicts with rowsum/colparts/nsq_own/dots_own/poi."""
    B, MT, JT, S = cfg.B, cfg.MT, cfg.JT, cfg.S
    nsq = np.zeros((3, B), np.float64)
    dots = np.zeros((3, B), np.float64)
    rowsum = np.zeros((3, B), np.float64)
    colsum = np.zeros((3, B), np.float64)
    poi_in = 0.0
    poi_tl = 0.0
    for c, r in enumerate(per_core):
        rs = np.asarray(r["rowsum"], np.float64)[0]    # [3*S]
        cp = np.asarray(r["colparts"], np.float64)     # [128, 3*JT]
        nq = np.asarray(r["nsq_own"], np.float64)      # [128, 3*MT]
        dt_ = np.asarray(r["dots_own"], np.float64)
        po = np.asarray(r["poi"], np.float64)          # [128, 2*MT]
        for fi in range(3):
            for t in range(MT):
                nsq[fi, c * S + t * P : c * S + (t + 1) * P] = nq[:, fi * MT + t]
        for pi in range(3):
            rowsum[pi, c * S : (c + 1) * S] = rs[pi * S : (pi + 1) * S]
            for t in range(MT):
                rows = slice(c * S + t * P, c * S + (t + 1) * P)
                dots[pi, rows] = dt_[:, pi * MT + t]
            for jt in range(JT):
                colsum[pi, jt * P : (jt + 1) * P] += cp[:, pi * JT + jt]
        poi_in += po[:, :MT].sum()
        poi_tl += po[:, MT:].sum()

    na = np.sqrt(nsq)  # [3, B]
    pairs = ((0, 1), (0, 2), (1, 2))
    closs = 0.0
    for pi, (ia, ib) in enumerate(pairs):
        simdiag = dots[pi] / (na[ia] * na[ib])
        loss_i = np.mean(np.log(rowsum[pi]) - simdiag / TEMPERATURE)
        loss_j = np.mean(np.log(colsum[pi]) - simdiag / TEMPERATURE)
        closs += 0.5 * (loss_i + loss_j)
    closs /= 3.0
    p_loss = (poi_in - poi_tl) / (cfg.B * cfg.D)
    total = p_loss + closs
    return (
        np.float32(total),
        np.float32(p_loss),
        np.float32(closs),
    )


_CACHE = {}


def _get_compiled(cfg: Cfg):
    key = (cfg.B, cfg.D, cfg.n_cores, cfg.fp8)
    if key not in _CACHE:
        _CACHE[key] = build_bass(cfg)
    return _CACHE[key]


def kernel(inputs, targets, feature1, feature2, feature3):
    from concourse.bass_utils import run_bass_kernel_spmd

    cfg = Cfg(B=inputs.shape[0], D=inputs.shape[1], n_cores=8)
    nc = _get_compiled(cfg)
    in_maps = make_in_maps(cfg, inputs, targets, feature1, feature2, feature3)
    res = run_bass_kernel_spmd(nc, in_maps, core_ids=list(range(cfg.n_cores)))
    return combine_results(cfg, res.results)


if __name__ == "__main__":
    rng = np.random.default_rng(0)
    B, D = 4096, 1024
    ins = {
        "inputs": rng.random((B, D), np.float32),
        "targets": rng.random((B, D), np.float32),
        "feature1": rng.standard_normal((B, D)).astype(np.float32),
        "feature2": rng.standard_normal((B, D)).astype(np.float32),
        "feature3": rng.standard_normal((B, D)).astype(np.float32),
    }
    out = kernel(**ins)
    print(out)


# revision 19
# speedup vs baseline: 2.0146x; 1.0548x over previous
"""Trainium2 Bass kernel for nn_CombinedLoss (Poisson + 3-way pairwise CLIP loss).

Strategy (8 NeuronCores, SPMD, one tiny AllGather):
  - Row-shard the batch: core c owns rows [c*512, (c+1)*512) of every tensor.
  - Similarity tiles are computed TRANSPOSED: for pair (a,b) the stationary
    matmul operand is the full feature b, host-transposed to [D, B] and cast
    to fp8/bf16 (a pure layout/dtype change -- all math stays on device); the
    moving operand is this core's own 512 rows of a, normalized on-device and
    PE-transposed.  out[j, i] = z_a[i] . b_raw[j].
  - With j on the PSUM partition axis, the 1/||b_j|| normalization folds into
    the exp's per-partition scale: exp(out * recipB_j / T).  Nobody ever
    normalizes (or even loads a row-major copy of) the full features.
  - recipB for all 4096 rows: each core computes squared norms of its OWN
    rows (DVE square-accum, nearly free) and a 4KB AllGather shares them.
  - Row sums of exp(sim/T): ones-matmul over the j-partition axis, PSUM-
    accumulated across all 32 j-tiles -> complete on device.
  - Column-sum partials: free via the exp activation's accum_out.
  - fp8 (e4m3) DoubleRow matmuls double PE throughput; fp32 accumulate.
  - Host does only the O(B) final combine: log of sums, means.
"""

import sys

import numpy as np

sys.path.insert(0, "/opt/trn_rl_repo")

P = 128
TEMPERATURE = 0.5
EPS_POISSON = 1e-8


class Cfg:
    def __init__(self, B=4096, D=1024, n_cores=8, fp8=True):
        self.B = B          # batch
        self.D = D          # feature dim
        self.n_cores = n_cores
        self.fp8 = fp8
        self.S = B // n_cores      # own rows per core
        self.MT = self.S // P      # own-row tiles (streaming free dim chunks)
        self.K = D // P            # contraction tiles
        self.JT = B // P           # stationary j tiles
        assert B % n_cores == 0 and self.S % P == 0 and D % P == 0
        if fp8:
            assert self.K % 2 == 0


def _patch_act_tables():
    """Make Bacc's act-table pass pick `natural_log_exp_and_others` for both
    Exp and Ln (they otherwise land in two different sets, and alternating
    Ln/Exp calls reload the 2.7us activation tables every tile)."""
    import functools

    import concourse.hw_specs as hw_specs

    if getattr(hw_specs, "_act_tables_patched", False):
        return
    orig = hw_specs.get_activation_tables

    @functools.cache
    def patched(module_arch):
        tabs = dict(orig(module_arch))
        names = list(tabs.keys())
        if "natural_log_exp_and_others" in tabs:
            combined = tabs["natural_log_exp_and_others"]
            for name in names:
                if name == "natural_log_exp_and_others":
                    break
                if tabs[name] & combined:
                    tabs[name] = tabs[name] - combined
        return tabs

    hw_specs.get_activation_tables = patched
    import concourse.bacc as bacc_mod

    if hasattr(bacc_mod, "get_activation_tables"):
        bacc_mod.get_activation_tables = patched
    hw_specs._act_tables_patched = True


def build_bass(cfg: Cfg):
    """Build the single-core Bass program (same program for all SPMD cores)."""
    import concourse.bacc as bacc
    import concourse.bass as bass
    import concourse.mybir as mybir
    import concourse.tile as tile
    from concourse.masks import make_identity

    _patch_act_tables()

    f32 = mybir.dt.float32
    bf16 = mybir.dt.bfloat16
    fp8 = mybir.dt.float8e4
    AF = mybir.ActivationFunctionType
    ALU = mybir.AluOpType
    ts = bass.ts

    B, D, K, MT, JT, S = cfg.B, cfg.D, cfg.K, cfg.MT, cfg.JT, cfg.S
    NC = cfg.n_cores
    mm_dt = fp8 if cfg.fp8 else bf16

    nc = bacc.Bacc(
        "TRN2",
        target_bir_lowering=False,
        debug=False,
        enable_asserts=False,
        num_devices=NC,
    )

    # ---- IO ----
    # fT2/fT3: full features 2,3 transposed [D, B], raw (unnormalized)
    fT2 = nc.dram_tensor("fT2", [D, B], mm_dt, kind="ExternalInput").ap()
    fT3 = nc.dram_tensor("fT3", [D, B], mm_dt, kind="ExternalInput").ap()
    # own row slices, bf16
    f1o = nc.dram_tensor("f1_own", [S, D], bf16, kind="ExternalInput").ap()
    f2o = nc.dram_tensor("f2_own", [S, D], bf16, kind="ExternalInput").ap()
    f3o = nc.dram_tensor("f3_own", [S, D], bf16, kind="ExternalInput").ap()
    inp = nc.dram_tensor("inp_own", [S, D], bf16, kind="ExternalInput").ap()
    tgt = nc.dram_tensor("tgt_own", [S, D], bf16, kind="ExternalInput").ap()

    rowsum_d = nc.dram_tensor("rowsum", [1, 3 * S], f32, kind="ExternalOutput").ap()
    colparts_d = nc.dram_tensor("colparts", [P, 3 * JT], f32, kind="ExternalOutput").ap()
    nsq_d = nc.dram_tensor("nsq_own", [P, 3 * MT], f32, kind="ExternalOutput").ap()
    dots_d = nc.dram_tensor("dots_own", [P, 3 * MT], f32, kind="ExternalOutput").ap()
    poi_d = nc.dram_tensor("poi", [P, 2 * MT], f32, kind="ExternalOutput").ap()

    own_dram = [f1o, f2o, f3o]
    fT_dram = [fT2, fT3]

    with tile.TileContext(nc) as tc:
        with (
            tc.tile_pool(name="const", bufs=1) as const_pool,
            tc.tile_pool(name="persist", bufs=1) as persist,
            tc.tile_pool(name="own", bufs=1) as ownp,
            tc.tile_pool(name="stage", bufs=2) as stage,
            tc.tile_pool(name="junk", bufs=2) as junkp,
            tc.tile_pool(name="exps", bufs=6) as expp,
            tc.tile_pool(name="small", bufs=8) as smallp,
            tc.tile_pool(name="dscr", bufs=1, space="DRAM") as dramp,
            tc.tile_pool(name="ps_s", bufs=4, space="PSUM") as ps_s,
            tc.tile_pool(name="ps_t", bufs=1, space="PSUM") as ps_t,
            tc.tile_pool(name="ps_r", bufs=3, space="PSUM") as ps_r,
        ):
            identity = const_pool.tile([P, P], bf16)
            make_identity(nc, identity)
            ones = const_pool.tile([P, 1], bf16)
            nc.vector.memset(ones, 1.0)
            eps_bias = const_pool.tile([P, 1], f32)
            nc.vector.memset(eps_bias, EPS_POISSON)

            # persistent state
            fT_sb = [persist.tile([P, K, B], mm_dt, name=f"fT_sb{i}") for i in range(2)]
            zT = [persist.tile([P, K, S], mm_dt, name=f"zT{i}") for i in range(2)]
            nsq_own = persist.tile([P, 3 * MT], f32)
            dots_own = persist.tile([P, 3 * MT], f32)
            poi = persist.tile([P, 2 * MT], f32)
            colparts = persist.tile([P, 3 * JT], f32)
            rowsum_sb = persist.tile([1, 3 * S], f32)
            scaleB = [persist.tile([P, JT], f32, name=f"scaleB{i}") for i in range(2)]
            # sim-stash for tiles computed before the gathered norms arrive
            N_STASH_F = min(6, JT)    # fused groups stashed (2 tiles each)
            stash12 = persist.tile([P, JT, S], bf16)
            stashF = persist.tile([P, 2 * N_STASH_F, S], bf16)

            own_rf = {}  # (fi, t) -> own bf16 row tile

            def rsqrt_act(dst, src, n, tag):
                # dst[:, :n] = 1/sqrt(src) = exp(-0.5*ln(src))
                l = smallp.tile([P, n], f32, tag=tag)
                nc.scalar.activation(l, src, AF.Ln)
                nc.scalar.activation(dst, l, AF.Exp, scale=-0.5)

            # ---- Phase A1: own loads + squared norms -> collective, ASAP ----
            # sync queue: f1o (zT critical path), f3o+f2o (collective), fT2.
            for fi in (0, 2, 1):
                for t in range(MT):
                    rf = ownp.tile([P, D], bf16, tag=f"own{fi}_{t}")
                    nc.sync.dma_start(rf, own_dram[fi][ts(t, P), :])
                    own_rf[(fi, t)] = rf
            for t in range(MT):  # f1 squares on DVE (first: feeds zT[0])
                jt_ = junkp.tile([P, D], bf16, tag="junk16")
                nc.vector.scalar_tensor_tensor(
                    out=jt_, in0=own_rf[(0, t)], scalar=1.0, in1=own_rf[(0, t)],
                    op0=ALU.mult, op1=ALU.mult,
                    accum_out=nsq_own[:, t : t + 1],
                )
            recip_own = smallp.tile([P, 2 * MT], f32, tag="recip_own")
            rsqrt_act(recip_own[:, :MT], nsq_own[:, :MT], MT, tag="ln_own1")
            for t in range(MT):  # f3 squares on ACT (collective input)
                jt_ = junkp.tile([P, D], bf16, tag="junk16")
                nc.scalar.activation(
                    jt_, own_rf[(2, t)], AF.Square,
                    accum_out=nsq_own[:, 2 * MT + t : 2 * MT + t + 1],
                )
            for t in range(MT):  # f2 squares on DVE (collective input)
                jt_ = junkp.tile([P, D], bf16, tag="junk16")
                nc.vector.scalar_tensor_tensor(
                    out=jt_, in0=own_rf[(1, t)], scalar=1.0, in1=own_rf[(1, t)],
                    op0=ALU.mult, op1=ALU.mult,
                    accum_out=nsq_own[:, MT + t : MT + t + 1],
                )
            cc_in = dramp.tile([P, 2 * MT], f32, name="cc_in")
            cc_out = dramp.tile([NC * P, 2 * MT], f32, name="cc_out")
            nc.gpsimd.dma_start(cc_in[:], nsq_own[:, MT : 3 * MT])
            nc.gpsimd.collective_compute(
                "AllGather", mybir.AluOpType.bypass,
                replica_groups=[list(range(NC))],
                ins=[cc_in.opt()], outs=[cc_out.opt()],
            )
            for k in range(K):  # fT2 on sync queue
                nc.sync.dma_start(fT_sb[0][:, k, :], fT_dram[0][ts(k, P), :])
            for k in range(K):  # fT3 on gpsimd queue
                nc.gpsimd.dma_start(fT_sb[1][:, k, :], fT_dram[1][ts(k, P), :])

            # ---- Phase A2: normalize + PE-transpose own rows into zT ----
            def make_zT(a, t):
                zrow = stage.tile([P, D], bf16, tag="zrow")
                nc.vector.tensor_scalar_mul(
                    zrow, own_rf[(a, t)], recip_own[:, a * MT + t : a * MT + t + 1]
                )
                tps = ps_t.tile([P, K * P], bf16, tag="tps")
                for k in range(K):
                    nc.tensor.transpose(tps[:, ts(k, P)], zrow[:, ts(k, P)], identity)
                nc.vector.tensor_copy(
                    out=zT[a][:, :, ts(t, P)],
                    in_=tps[:].rearrange("p (k c) -> p k c", k=K),
                )

            for t in range(MT):
                make_zT(0, t)
            rsqrt_act(recip_own[:, MT:], nsq_own[:, MT : 2 * MT], MT, tag="ln_own2")
            for t in range(MT):
                make_zT(1, t)

            for pi, (ia, ib) in enumerate(((0, 1), (0, 2), (1, 2))):
                for t in range(MT):
                    jt_ = junkp.tile([P, D], bf16, tag="junk16")
                    nc.vector.scalar_tensor_tensor(
                        out=jt_, in0=own_rf[(ia, t)], scalar=1.0, in1=own_rf[(ib, t)],
                        op0=ALU.mult, op1=ALU.mult,
                        accum_out=dots_own[:, pi * MT + t : pi * MT + t + 1],
                    )

            # ---- Phase B: main matmuls ----
            # orientation: out[j, i] = fT_b[:, j] . zT_a[:, i]; stationary fT_b.
            def mm_group(b, a, ps, jt):
                if cfg.fp8:
                    for kk in range(0, K, 2):
                        nc.tensor.matmul(
                            ps,
                            fT_sb[b][:, kk : kk + 2, ts(jt, P)],
                            zT[a][:, kk : kk + 2, :],
                            start=(kk == 0), stop=(kk == K - 2),
                            perf_mode=mybir.MatmulPerfMode.DoubleRow,
                            skip_group_check=True,
                        )
                else:
                    for k in range(K):
                        nc.tensor.matmul(
                            ps,
                            fT_sb[b][:, k, ts(jt, P)],
                            zT[a][:, k, :],
                            start=(k == 0), stop=(k == K - 1),
                            skip_group_check=True,
                        )

            def mm_group_fused(ps13, ps23, jt):
                # pairs (f1,f3) and (f2,f3) share stationary fT3 -> one
                # LDWEIGHTS per (jt, kk) serves two matmuls.
                if cfg.fp8:
                    for kk in range(0, K, 2):
                        w = fT_sb[1][:, kk : kk + 2, ts(jt, P)]
                        for ps, a in ((ps13, 0), (ps23, 1)):
                            nc.tensor.matmul(
                                ps, w, zT[a][:, kk : kk + 2, :],
                                start=(kk == 0), stop=(kk == K - 2),
                                perf_mode=mybir.MatmulPerfMode.DoubleRow,
                                skip_group_check=True,
                            )
                else:
                    for k in range(K):
                        w = fT_sb[1][:, k, ts(jt, P)]
                        for ps, a in ((ps13, 0), (ps23, 1)):
                            nc.tensor.matmul(
                                ps, w, zT[a][:, k, :],
                                start=(k == 0), stop=(k == K - 1),
                                skip_group_check=True,
                            )

            def do_exp(pi, b, jt, src):
                # es = exp(src * recipB_j / T); column-sum partial via DVE
                es = expp.tile([P, S], bf16, tag="exps")
                nc.scalar.activation(
                    es, src, AF.Exp, scale=scaleB[b][:, jt : jt + 1]
                )
                slot = pi * JT + jt
                nc.vector.tensor_reduce(
                    out=colparts[:, slot : slot + 1], in_=es,
                    axis=mybir.AxisListType.X, op=ALU.add,
                )
                return es

            es_tiles = {}  # (pi, jt) -> es tile

            def build_scaleB():
                # gathered norms -> per-partition exp scales (collective
                # result); emitted mid-M2 so the in-order ACT stream does
                # not block earlier stash copies on the collective.
                nsqB = smallp.tile([P, NC, 2 * MT], f32, tag="nsqB")
                nc.sync.dma_start(
                    nsqB, cc_out[:].rearrange("(r p) m -> p r m", r=NC)
                )
                for b in range(2):
                    lnB = smallp.tile([P, JT], f32, tag=f"lnB{b}")
                    nc.scalar.activation(
                        lnB[:].rearrange("p (r m) -> p r m", r=NC),
                        nsqB[:, :, b * MT : (b + 1) * MT],
                        AF.Ln,
                    )
                    recipB = smallp.tile([P, JT], f32, tag=f"recipB{b}")
                    nc.scalar.activation(recipB, lnB, AF.Exp, scale=-0.5)
                    nc.vector.tensor_scalar_mul(
                        scaleB[b], recipB, 1.0 / TEMPERATURE
                    )

            # Row-sum accumulators: one PSUM bank per pair, accumulated by
            # ones-matmuls in whatever order the exp tiles become available.
            rs_ps = {pi: ps_r.tile([1, S], f32, tag="ps_r", name=f"rs{pi}")
                     for pi in range(3)}
            rs_count = {0: 0, 1: 0, 2: 0}

            def emit_rowsum(pi, jt):
                n = rs_count[pi]
                nc.tensor.matmul(rs_ps[pi], ones, es_tiles[(pi, jt)],
                                 start=(n == 0), stop=(n == JT - 1),
                                 skip_group_check=True)
                rs_count[pi] = n + 1

            def stash_src(pi, jt):
                if pi == 0:
                    return stash12[:, jt, :]
                return stashF[:, 2 * jt + (pi - 1), :]

            # stash items in the order they will be exp'd once scaleB lands
            stash_items = [(0, 0, jt) for jt in range(JT)]
            stash_items += [(pi, 1, jt) for jt in range(min(N_STASH_F, JT))
                            for pi in (1, 2)]
            CHUNK = 2
            rs_ready = []  # stash items exp'd last group, rowsum pending

            # M1: pair (f1,f2): all groups stashed (gathered norms not
            # ready yet); copies run on otherwise-idle ACT.
            for jt in range(JT):
                ps = ps_s.tile([P, S], f32, tag="ps_s")
                mm_group(0, 0, ps, jt)
                nc.scalar.activation(stash12[:, jt, :], ps, AF.Copy)

            # M2: fused pairs (f1,f3), (f2,f3): first N_STASH_F groups
            # stashed, rest exp'd directly from PSUM; stash-exps and
            # row-sums spread across the direct groups with a lag.
            RLAG = 2
            for jt in range(JT):
                ps13 = ps_s.tile([P, S], f32, tag="ps_s")
                ps23 = ps_s.tile([P, S], f32, tag="ps_s")
                mm_group_fused(ps13, ps23, jt)
                g = jt - RLAG
                if g >= N_STASH_F:
                    emit_rowsum(1, g)
                    emit_rowsum(2, g)
                for pi_s, jt_s in rs_ready:
                    emit_rowsum(pi_s, jt_s)
                rs_ready = []
                if jt < N_STASH_F:
                    nc.scalar.activation(stashF[:, 2 * jt, :], ps13, AF.Copy)
                    nc.scalar.activation(stashF[:, 2 * jt + 1, :], ps23, AF.Copy)
                else:
                    if jt == N_STASH_F:
                        build_scaleB()
                    es_tiles[(1, jt)] = do_exp(1, 1, jt, ps13)
                    es_tiles[(2, jt)] = do_exp(2, 1, jt, ps23)
                    for _ in range(CHUNK):
                        if stash_items:
                            pi_s, b_s, jt_s = stash_items.pop(0)
                            es_tiles[(pi_s, jt_s)] = do_exp(
                                pi_s, b_s, jt_s, stash_src(pi_s, jt_s))
                            rs_ready.append((pi_s, jt_s))
            if N_STASH_F >= JT:
                build_scaleB()

            # M3: drain whatever is left
            for pi_s, jt_s in rs_ready:
                emit_rowsum(pi_s, jt_s)
            for pi_s, b_s, jt_s in stash_items:
                es_tiles[(pi_s, jt_s)] = do_exp(pi_s, b_s, jt_s,
                                                stash_src(pi_s, jt_s))
                emit_rowsum(pi_s, jt_s)
            for g in range(max(N_STASH_F, JT - RLAG), JT):
                emit_rowsum(1, g)
                emit_rowsum(2, g)
            assert rs_count[0] == rs_count[1] == rs_count[2] == JT
            for pi in range(3):
                nc.any.tensor_copy(
                    out=rowsum_sb[:, pi * S : (pi + 1) * S], in_=rs_ps[pi])

            # ---- Phase C: poisson tail ----
            for t in range(MT):
                it = stage.tile([P, D], bf16, tag="it")
                tt = stage.tile([P, D], bf16, tag="tt")
                nc.sync.dma_start(it, inp[ts(t, P), :])
                nc.sync.dma_start(tt, tgt[ts(t, P), :])
                lg = stage.tile([P, D], f32, tag="lg")
                nc.scalar.activation(lg, it, AF.Ln, bias=eps_bias[:, :])
                jt_ = junkp.tile([P, D], bf16, tag="junk16")
                nc.vector.scalar_tensor_tensor(
                    out=jt_, in0=tt, scalar=1.0, in1=lg,
                    op0=ALU.mult, op1=ALU.mult,
                    accum_out=poi[:, MT + t : MT + t + 1],
                )
                jt2 = junkp.tile([P, D], bf16, tag="junk16")
                nc.vector.tensor_scalar(
                    out=jt2, in0=it, scalar1=1.0, scalar2=0.0, op0=ALU.mult,
                    op1=ALU.add, accum_out=poi[:, t : t + 1],
                )

            # ---- outputs ----
            nc.gpsimd.dma_start(rowsum_d, rowsum_sb)
            nc.gpsimd.dma_start(colparts_d, colparts)
            nc.gpsimd.dma_start(nsq_d, nsq_own)
            nc.gpsimd.dma_start(dots_d, dots_own)
            nc.gpsimd.dma_start(poi_d, poi)

    nc.compile()
    return nc


def make_in_maps(cfg: Cfg, inputs, targets, feature1, feature2, feature3):
    import ml_dtypes

    bf16 = ml_dtypes.bfloat16
    mm_np = ml_dtypes.float8_e4m3 if cfg.fp8 else bf16
    ac = np.ascontiguousarray

    # shared across cores: full transposed raw features (layout+dtype only)
    fT2 = ac(feature2.T).astype(mm_np)
    fT3 = ac(feature3.T).astype(mm_np)
    f1b = feature1.astype(bf16)
    f2b = feature2.astype(bf16)
    f3b = feature3.astype(bf16)
    inb = inputs.astype(bf16)
    tgb = targets.astype(bf16)

    maps = []
    for c in range(cfg.n_cores):
        sl = slice(c * cfg.S, (c + 1) * cfg.S)
        maps.append({
            "fT2": fT2,
            "fT3": fT3,
            "f1_own": ac(f1b[sl]),
            "f2_own": ac(f2b[sl]),
            "f3_own": ac(f3b[sl]),
            "inp_own": ac(inb[sl]),
            "tgt_own": ac(tgb[sl]),
        })
    return maps


def combine_results(cfg: Cfg, per_core):
    """per_core: list of dicts with rowsum/colparts/nsq_own/dots_own/poi."""
    B, MT, JT, S = cfg.B, cfg.MT, cfg.JT, cfg.S
    nsq = np.zeros((3, B), np.float64)
    dots = np.zeros((3, B), np.float64)
    rowsum = np.zeros((3, B), np.float64)
    colsum = np.zeros((3, B), np.float64)
    poi_in = 0.0
    poi_tl = 0.0
    for c, r in enumerate(per_core):
        rs = np.asarray(r["rowsum"], np.float64)[0]    # [3*S]
        cp = np.asarray(r["colparts"], np.float64)     # [128, 3*JT]
        nq = np.asarray(r["nsq_own"], np.float64)      # [128, 3*MT]
        dt_ = np.asarray(r["dots_own"], np.float64)
        po = np.asarray(r["poi"], np.float64)          # [128, 2*MT]
        for fi in range(3):
            for t in range(MT):
                nsq[fi, c * S + t * P : c * S + (t + 1) * P] = nq[:, fi * MT + t]
        for pi in range(3):
            rowsum[pi, c * S : (c + 1) * S] = rs[pi * S : (pi + 1) * S]
            for t in range(MT):
                rows = slice(c * S + t * P, c * S + (t + 1) * P)
                dots[pi, rows] = dt_[:, pi * MT + t]
            for jt in range(JT):
                colsum[pi, jt * P : (jt + 1) * P] += cp[:, pi * JT + jt]
        poi_in += po[:, :MT].sum()
        poi_tl += po[:, MT:].sum()

    na = np.sqrt(nsq)  # [3, B]
    pairs = ((0, 1), (0, 2), (1, 2))
    closs = 0.0
    for pi, (ia, ib) in enumerate(pairs):
        simdiag = dots[pi] / (na[ia] * na[ib])
        loss_i = np.mean(np.log(rowsum[pi]) - simdiag / TEMPERATURE)
        loss_j = np.mean(np.log(colsum[pi]) - simdiag / TEMPERATURE)
        closs += 0.5 * (loss_i + loss_j)
    closs /= 3.0
    p_loss = (poi_in - poi_tl) / (cfg.B * cfg.D)
    total = p_loss + closs
    return (
        np.float32(total),
        np.float32(p_loss),
        np.float32(closs),
    )


_CACHE = {}


def _get_compiled(cfg: Cfg):
    key = (cfg.B, cfg.D, cfg.n_cores, cfg.fp8)
    if key not in _CACHE:
        _CACHE[key] = build_bass(cfg)
    return _CACHE[key]


def kernel(inputs, targets, feature1, feature2, feature3):
    from concourse.bass_utils import run_bass_kernel_spmd

    cfg = Cfg(B=inputs.shape[0], D=inputs.shape[1], n_cores=8)
    nc = _get_compiled(cfg)
    in_maps = make_in_maps(cfg, inputs, targets, feature1, feature2, feature3)
    res = run_bass_kernel_spmd(nc, in_maps, core_ids=list(range(cfg.n_cores)))
    return combine_results(cfg, res.results)


if __name__ == "__main__":
    rng = np.random.default_rng(0)
    B, D = 4096, 1024
    ins = {
        "inputs": rng.random((B, D), np.float32),
        "targets": rng.random((B, D), np.float32),
        "feature1": rng.standard_normal((B, D)).astype(np.float32),
        "feature2": rng.standard_normal((B, D)).astype(np.float32),
        "feature3": rng.standard_normal((B, D)).astype(np.float32),
    }
    out = kernel(**ins)
    print(out)
